# revision 44
# baseline (speedup 1.0000x reference)
"""Trainium2 Bass kernel for nn_Autocorrelation (B=16, L=1024, D=512, H=8, dh=64).

Self-contained: kernel(**inputs) -> np.ndarray [16, 1024, 512] float32.

Design notes:
- The reference broadcasts ONE projection across all 8 heads, so the real work
  is per (batch, dh) row: 16*64 = 1024 rows of length L=1024. Each core takes
  2 batches = 128 rows (exactly the SBUF partition count). Head replication is
  a pure broadcast, done on the host (np.tile) from a compact [B, L, 64] out.
- FFT circular correlation is replaced by dense DFT matmuls against cos/sin
  matrices; real-input symmetry folds the spectrum to f in [0, 640) (5x128
  chunks) with per-frequency weights alpha (1, 2...2, 1, 0...0) applied for
  free inside the spectral chunk-transpose copies (per-partition scalars).
- ifft's 1/L is folded into the q projection (Wq/L, bq/L) for stage 1 and into
  the masked-softmax weights (w/L) for stage 2, so corr keeps reference scale.
- top-13 is selection only: t13 = 13th largest via two DVE max8 rounds +
  match_replace; s = exp(corr-max)*(corr>=t13)/masked_sum * (1/L). No indices.
- The lag-gather + weighted sum is a second circular correlation with the
  sparse weight row s.
- Precision: fp16 operands (10-bit mantissa) with f32 PSUM accumulation keep
  corr error ~1e-3 relative, small enough that top-k selection swaps are rare;
  measured end-to-end rel err ~4e-3 (gate 2e-2). Inputs/constants are host-cast
  to fp16 and host-pre-transposed to [B, D, L] so all loads are contiguous.
"""

import threading

import numpy as np

L = 1024
D = 512
DH = 64
BLOC = 2          # batches per core
B = 16
H = 8
KTOP = 13
NCORES = 8
F = 640           # rfft fold: f in [0, 640), alpha-weighted
FC = 5

F32MAX_NEG = -1.0e30


def _build_nc(cfg=None):
    from contextlib import ExitStack

    import concourse.bass as bass
    import concourse.mybir as mybir
    import concourse.tile as tile
    from concourse import bacc
    from concourse.masks import make_identity

    f32 = mybir.dt.float32
    f16 = mybir.dt.float16
    AF = mybir.ActivationFunctionType
    ALU = mybir.AluOpType

    nc = bacc.Bacc("TRN2", target_bir_lowering=False, debug=False, num_devices=NCORES)

    # pre-transposed fp16 inputs [BLOC, D, L]
    Qf = nc.declare_dram_parameter("Qf", [BLOC, D, L], f16, isOutput=False)
    Kf = nc.declare_dram_parameter("Kf", [BLOC, D, L], f16, isOutput=False)
    Vf = nc.declare_dram_parameter("Vf", [BLOC, D, L], f16, isOutput=False)
    Wlf = nc.declare_dram_parameter("Wlf", [D, DH], f16, isOutput=False)   # Wq/L
    blf = nc.declare_dram_parameter("blf", [DH], f32, isOutput=False)      # bq/L
    Wuf = nc.declare_dram_parameter("Wuf", [D, DH], f16, isOutput=False)
    buf_ = nc.declare_dram_parameter("buf", [DH], f32, isOutput=False)
    Cm = nc.declare_dram_parameter("Cm", [L, L], f16, isOutput=False)
    Sm = nc.declare_dram_parameter("Sm", [L, L], f16, isOutput=False)
    Al = nc.declare_dram_parameter("Al", [F], f32, isOutput=False)
    outd = nc.declare_dram_parameter("out", [BLOC, L, DH], f32, isOutput=True)

    with tile.TileContext(nc) as tc, ExitStack() as ctx:
        consts = ctx.enter_context(tc.tile_pool(name="consts", bufs=1))
        rowsp = ctx.enter_context(tc.tile_pool(name="rowsp", bufs=1))
        xpool = ctx.enter_context(tc.tile_pool(name="xpool", bufs=8))
        pjp = ctx.enter_context(tc.tile_pool(name="pjp", bufs=4))
        spec = ctx.enter_context(tc.tile_pool(name="spec", bufs=6))
        rowbig = ctx.enter_context(tc.tile_pool(name="rowbig", bufs=6))
        tmpbig = ctx.enter_context(tc.tile_pool(name="tmpbig", bufs=6))
        chunksp = ctx.enter_context(tc.tile_pool(name="chunksp", bufs=4))
        small = ctx.enter_context(tc.tile_pool(name="small", bufs=2))
        atp = ctx.enter_context(tc.tile_pool(name="atp", bufs=4))
        psum_big = ctx.enter_context(
            tc.tile_pool(name="psum_big", bufs=4, space="PSUM")
        )
        psum_t = ctx.enter_context(tc.tile_pool(name="psum_t", bufs=4, space="PSUM"))

        # ---- small constants ----
        identh = consts.tile([128, 128], f16)
        make_identity(nc, identh)
        identf = consts.tile([128, 128], f32)
        make_identity(nc, identf)
        Wl_sb = consts.tile([128, 4, DH], f16)
        nc.sync.dma_start(out=Wl_sb, in_=Wlf[:, :].rearrange("(c p) h -> p c h", p=128))
        Wu_sb = consts.tile([128, 4, DH], f16)
        nc.sync.dma_start(out=Wu_sb, in_=Wuf[:, :].rearrange("(c p) h -> p c h", p=128))

        def as_col(ap):
            return bass.AP(tensor=ap.tensor, offset=ap.offset,
                           ap=list(ap.ap) + [[0, 1]])

        bl_p = consts.tile([DH, 1], f32)
        nc.gpsimd.dma_start(out=bl_p, in_=as_col(blf[:]))
        bu_p = consts.tile([DH, 1], f32)
        nc.gpsimd.dma_start(out=bu_p, in_=as_col(buf_[:]))
        alpha = consts.tile([128, FC], f32)
        nc.sync.dma_start(out=alpha, in_=Al[:].rearrange("(c p) -> p c", p=128))
        warm = small.tile([128, 1], f32, tag="warm", bufs=1)
        nc.gpsimd.memset(warm, 0.0)
        nc.scalar.activation(warm, warm, AF.Exp, bias=0.0, scale=1.0)

        # ---- input path: contiguous loads of [128(d), b*L] chunks, single
        # fp16 projection (W stationary, x moving 512), PE-transpose into
        # rows[128(j), jc, 64*b+dh] ----
        rows_q = rowsp.tile([128, 8, 128], f16)
        rows_k = rowsp.tile([128, 8, 128], f16)
        rows_v = rowsp.tile([128, 8, 128], f16)
        plan = (
            (Qf, Wl_sb, bl_p, rows_q),
            (Kf, Wu_sb, bu_p, rows_k),
            (Vf, Wu_sb, bu_p, rows_v),
        )
        for ti, (xd, Wsb, bias_p, rows_dst) in enumerate(plan):
            pss = {}
            for b in range(BLOC):
                pss[(b, 0)] = psum_big.tile([DH, 512], f32, tag="big", name=f"pj{ti}{b}0")
                pss[(b, 1)] = psum_big.tile([DH, 512], f32, tag="big", name=f"pj{ti}{b}1")
            for dc in range(4):
                th = xpool.tile([128, 2 * L], f16, tag="th")
                nc.sync.dma_start(
                    out=th.rearrange("p (b l) -> p b l", b=BLOC),
                    in_=xd[:, dc * 128 : (dc + 1) * 128, :].rearrange(
                        "b d l -> d b l"
                    ),
                )
                st, sp = dc == 0, dc == 3
                for b in range(BLOC):
                    for hh in range(2):
                        sl = slice(b * L + hh * 512, b * L + (hh + 1) * 512)
                        nc.tensor.matmul(pss[(b, hh)], lhsT=Wsb[:, dc, :],
                                         rhs=th[:, sl], start=st, stop=sp)
            for b in range(BLOC):
                projT = pjp.tile([DH, L], f16, tag="pj_t")
                nc.vector.tensor_scalar(projT[:, 0:512], pss[(b, 0)], scalar1=bias_p,
                                        scalar2=None, op0=ALU.add)
                nc.vector.tensor_scalar(projT[:, 512:1024], pss[(b, 1)], scalar1=bias_p,
                                        scalar2=None, op0=ALU.add)
                for lt in range(8):
                    tp = psum_t.tile([128, DH], f16, tag="tr")
                    nc.tensor.transpose(
                        tp, projT[:, lt * 128 : (lt + 1) * 128], identh[:DH, :DH]
                    )
                    nc.vector.tensor_copy(rows_dst[:, lt, 64 * b : 64 * b + 64], tp)

        # ---- big constants: forward C/S [j, f<640], inverse C/S [f<640, tau]
        Cfw = consts.tile([128, 8, F], f16)
        nc.sync.dma_start(out=Cfw, in_=Cm[:, 0:F].rearrange("(a p) x -> p a x", p=128))
        Sfw = consts.tile([128, 8, F], f16)
        nc.sync.dma_start(out=Sfw, in_=Sm[:, 0:F].rearrange("(a p) x -> p a x", p=128))
        Cin = consts.tile([128, FC, L], f16)
        nc.sync.dma_start(out=Cin, in_=Cm[0:F, :].rearrange("(a p) x -> p a x", p=128))
        Sin = consts.tile([128, FC, L], f16)
        nc.sync.dma_start(out=Sin, in_=Sm[0:F, :].rearrange("(a p) x -> p a x", p=128))

        # ---- forward DFT: rows [128(j), jc, 128(r)] -> spectra [128(r), F] ----
        def fwd(rows_src, nm):
            re_ = spec.tile([128, F], f16, tag="spec", name=f"{nm}_re")
            im_ = spec.tile([128, F], f16, tag="spec", name=f"{nm}_im")
            pr0 = psum_big.tile([128, 320], f32, tag="big")
            pr1 = psum_big.tile([128, 320], f32, tag="big")
            pi0 = psum_big.tile([128, 320], f32, tag="big")
            pi1 = psum_big.tile([128, 320], f32, tag="big")
            for jc in range(8):
                lhsT = rows_src[:, jc, :]
                st, sp = jc == 0, jc == 7
                nc.tensor.matmul(pr0, lhsT=lhsT, rhs=Cfw[:, jc, 0:320], start=st, stop=sp)
                nc.tensor.matmul(pr1, lhsT=lhsT, rhs=Cfw[:, jc, 320:640], start=st, stop=sp)
                nc.tensor.matmul(pi0, lhsT=lhsT, rhs=Sfw[:, jc, 0:320], start=st, stop=sp)
                nc.tensor.matmul(pi1, lhsT=lhsT, rhs=Sfw[:, jc, 320:640], start=st, stop=sp)
            nc.any.tensor_copy(re_[:, 0:320], pr0)
            nc.any.tensor_copy(re_[:, 320:640], pr1)
            nc.any.tensor_copy(im_[:, 0:320], pi0)
            nc.any.tensor_copy(im_[:, 320:640], pi1)
            return re_, im_

        # ---- transpose row tensor into chunk layout [128, c, 128]; alpha is
        # folded into the PSUM->SBUF copy as a per-partition scalar ----
        def to_chunks(src, nm, nch, al_col=None):
            idt = identf if src.dtype == f32 else identh
            dstT = chunksp.tile([128, nch, 128], f16, tag=f"chT{nch}", name=f"{nm}_T")
            for fc in range(nch):
                tp = psum_t.tile([128, 128], src.dtype, tag="tr")
                nc.tensor.transpose(tp, src[:, fc * 128 : (fc + 1) * 128], idt)
                if al_col is not None:
                    nc.any.tensor_scalar(dstT[:, fc, :], tp,
                                         scalar1=al_col[:, fc : fc + 1],
                                         scalar2=None, op0=ALU.mult)
                else:
                    nc.any.tensor_copy(dstT[:, fc, :], tp)
            return dstT

        # ---- inverse DFT: res[r, tau] = sum_f ReT[f,r]*C[f,tau] + ImT*S ----
        def inv(ReT, ImT, nm):
            res = rowbig.tile([128, L], f32, tag="row", name=f"{nm}_res")
            for hh in range(2):
                ps = psum_big.tile([128, 512], f32, tag="big")
                sl = slice(hh * 512, (hh + 1) * 512)
                for fc in range(FC):
                    st, sp = fc == 0, fc == FC - 1
                    nc.tensor.matmul(ps, lhsT=ReT[:, fc, :], rhs=Cin[:, fc, sl], start=st, stop=False)
                    nc.tensor.matmul(ps, lhsT=ImT[:, fc, :], rhs=Sin[:, fc, sl], start=False, stop=sp)
                nc.any.tensor_copy(res[:, sl], ps)
            return res

        Qr, Qi = fwd(rows_q, "q")
        Kr, Ki = fwd(rows_k, "k")

        # pointwise: X = Qhat * conj(Khat)   (f32 on DVE)
        t1 = tmpbig.tile([128, F], f16, tag="tmp")
        t2 = tmpbig.tile([128, F], f16, tag="tmp")
        nc.vector.tensor_mul(t1, Qr, Kr)
        nc.vector.tensor_mul(t2, Qi, Ki)
        XR = rowbig.tile([128, F], f16, tag="rowh")
        nc.vector.tensor_add(XR, t1, t2)
        t3 = tmpbig.tile([128, F], f16, tag="tmp")
        t4 = tmpbig.tile([128, F], f16, tag="tmp")
        nc.vector.tensor_mul(t3, Qi, Kr)
        nc.vector.tensor_mul(t4, Qr, Ki)
        XI = rowbig.tile([128, F], f16, tag="rowh")
        nc.vector.tensor_sub(XI, t3, t4)

        XRT = to_chunks(XR, "xr", FC, alpha)
        XIT = to_chunks(XI, "xi", FC, alpha)
        corr = inv(XRT, XIT, "corr")

        # forward DFT of v early (fills the PE while top-k runs on DVE)
        Vr, Vi = fwd(rows_v, "v")

        # ---- top-13 via masked softmax (no indices needed) ----
        vals16 = small.tile([128, 16], f32, tag="vals")
        corr2 = rowbig.tile([128, L], f32, tag="row")
        nc.vector.max(out=vals16[:, 0:8], in_=corr)
        nc.vector.match_replace(
            out=corr2, in_to_replace=vals16[:, 0:8], in_values=corr,
            imm_value=F32MAX_NEG,
        )
        nc.vector.max(out=vals16[:, 8:16], in_=corr2)
        negm = small.tile([128, 1], f32, tag="negm")
        nc.vector.tensor_scalar_mul(negm, vals16[:, 0:1], -1.0)
        ecorr = rowbig.tile([128, L], f32, tag="row")
        nc.scalar.activation(ecorr, corr, AF.Exp, bias=negm, scale=1.0)
        mask = tmpbig.tile([128, L], f32, tag="tmpL")
        nc.vector.tensor_scalar(
            mask, corr, scalar1=vals16[:, 12:13], scalar2=None, op0=ALU.is_ge,
        )
        em = rowbig.tile([128, L], f32, tag="row")
        nc.vector.tensor_mul(em, ecorr, mask)
        ssum = small.tile([128, 1], f32, tag="ssum")
        nc.vector.reduce_sum(ssum, em, axis=mybir.AxisListType.X)
        rs = small.tile([128, 1], f32, tag="rs")
        nc.vector.reciprocal(rs, ssum)
        s_t = rowbig.tile([128, L], f32, tag="row")
        nc.vector.tensor_scalar(
            s_t, em, scalar1=rs, scalar2=1.0 / L, op0=ALU.mult, op1=ALU.mult,
        )

        sT = to_chunks(s_t, "s", 8)
        Sr, Si = fwd(sT, "sp")

        # pointwise: Y = Vhat * conj(Shat)
        u1 = tmpbig.tile([128, F], f16, tag="tmp")
        u2 = tmpbig.tile([128, F], f16, tag="tmp")
        nc.vector.tensor_mul(u1, Vr, Sr)
        nc.vector.tensor_mul(u2, Vi, Si)
        YR = rowbig.tile([128, F], f16, tag="rowh")
        nc.vector.tensor_add(YR, u1, u2)
        u3 = tmpbig.tile([128, F], f16, tag="tmp")
        u4 = tmpbig.tile([128, F], f16, tag="tmp")
        nc.vector.tensor_mul(u3, Vi, Sr)
        nc.vector.tensor_mul(u4, Vr, Si)
        YI = rowbig.tile([128, F], f16, tag="rowh")
        nc.vector.tensor_sub(YI, u3, u4)

        YRT = to_chunks(YR, "yr", FC, alpha)
        YIT = to_chunks(YI, "yi", FC, alpha)

        # ---- inverse-2 directly transposed: aggT[tau, r] per tau-tile with
        # Cin/Sin as stationary operands; out-DMAs pipeline per tile ----
        for tt in range(8):
            ps = psum_big.tile([128, 128], f32, tag="big", name=f"po{tt}")
            tsl = slice(tt * 128, (tt + 1) * 128)
            for fc in range(FC):
                st, sp = fc == 0, fc == FC - 1
                nc.tensor.matmul(ps, lhsT=Cin[:, fc, tsl], rhs=YRT[:, fc, :],
                                 start=st, stop=False)
                nc.tensor.matmul(ps, lhsT=Sin[:, fc, tsl], rhs=YIT[:, fc, :],
                                 start=False, stop=sp)
            at = atp.tile([128, 128], f32, tag="at")
            nc.vector.tensor_copy(at, ps)
            for b in range(BLOC):
                nc.sync.dma_start(
                    out=outd[b, tsl, :],
                    in_=at[:, 64 * b : 64 * b + 64],
                )

    nc.compile()
    return nc


_cache = threading.Lock(), {}


def _get_nc():
    lock, store = _cache
    with lock:
        if "nc" not in store:
            store["nc"] = _build_nc()
        return store["nc"]


def _make_consts():
    j = np.arange(L, dtype=np.float64)
    ang = 2.0 * np.pi * np.outer(j, j) / L
    Cmat = np.cos(ang).astype(np.float16)
    Smat = (-np.sin(ang)).astype(np.float16)
    return Cmat, Smat


def _make_in_maps(Q, K, V, Wq, bq):
    Q = np.ascontiguousarray(Q, np.float32)
    K = np.ascontiguousarray(K, np.float32)
    V = np.ascontiguousarray(V, np.float32)
    Wq = np.ascontiguousarray(Wq, np.float32)
    bq = np.ascontiguousarray(bq, np.float32)

    def tr16(x):
        # [B, L, D] f32 -> [B, D, L] fp16 contiguous
        return np.ascontiguousarray(np.swapaxes(x, 1, 2).astype(np.float16))

    Qt, Kt, Vt = tr16(Q), tr16(K), tr16(V)
    Cmat, Smat = _make_consts()
    Wl16 = (Wq / L).astype(np.float16)
    Wu16 = Wq.astype(np.float16)
    bl32 = (bq / L).astype(np.float32)
    Alv = np.zeros(F, np.float32)
    Alv[0] = 1.0
    Alv[1 : L // 2] = 2.0
    Alv[L // 2] = 1.0
    in_maps = []
    for c in range(NCORES):
        sl = slice(BLOC * c, BLOC * (c + 1))
        in_maps.append(
            {
                "Qf": Qt[sl], "Kf": Kt[sl], "Vf": Vt[sl],
                "Wlf": Wl16, "blf": bl32, "Wuf": Wu16, "buf": bq,
                "Cm": Cmat, "Sm": Smat, "Al": Alv,
            }
        )
    return in_maps


def kernel(Q, K, V, Wq, bq):
    from concourse.bass_utils import run_bass_kernel_spmd

    nc = _get_nc()
    in_maps = _make_in_maps(Q, K, V, Wq, bq)
    res = run_bass_kernel_spmd(nc, in_maps, list(range(NCORES)))
    compact = np.concatenate([res.results[i]["out"] for i in range(NCORES)], axis=0)
    return np.tile(compact, (1, 1, H))
